# revision 69
# baseline (speedup 1.0000x reference)
"""Multi-head attention (B=2, S=2048, D=1024, H=16) on 8 Trainium2 cores.

Sharding: core i -> batch i//4, head-group i%4 (4 heads = 2 pairs of 2).
v6: the ACT-engine exp (128 x [128,1024], ~1.04us each) is the hard
floor of the attention phase, so everything is scheduled around keeping
ACT 100% busy from the earliest possible moment:

- ONE set of PSUM pools (scores 2x2 banks, accum 1x2, ring 2x1) lives
  for the whole kernel; the q/k projections and the v projection borrow
  them through tag rotation (a [128,1024] slot holds the cc0|cc1 pair
  of one seq-quarter, evacuated by ACT and DVE in parallel), so there
  are no pool-transition barriers anywhere near the critical path and
  the first exp fires ~30us, right after the k projection lands at the
  DMA roofline (stream order xq -> xk -> xv).
- the v projection (needs the full xv stream, which lands ~10us after
  exp starts) is woven into block-0's score/exp chunk stream; the
  deferred attnV work catches up through a work queue drained at a
  bounded rate per chunk so scores (and therefore ACT) never stall.
- attnV is flipped: out[q,65] += P_chunk[k,q]^T @ vaug[k,65] in bf16
  (1 cycle/row at 65-wide), accumulating 16 key chunks into one 2-bank
  PSUM tile per block; the Z column rides along via the ones-column of
  vaug; normalization is a per-partition broadcast on DVE; the
  normalized block is transposed back to [ch,pos] with a PE identity
  matmul; the output projection accumulates BOTH head pairs in one PSUM
  pass and stores bf16, so each core writes one bf16 [2048,1024]
  partial (host sums 8 + bias).
"""

import sys

import numpy as np

try:
    import concourse.bacc as bacc
except ImportError:  # grading dir may not have the repo on sys.path
    sys.path.insert(0, "/opt/trn_rl_repo")
    import concourse.bacc as bacc

import ml_dtypes
import concourse.masks as masks
import concourse.mybir as mybir
import concourse.tile as tile
from concourse import bass_utils

B, S, D, H, DH = 2, 2048, 1024, 16, 64
F32 = mybir.dt.float32
XDT = mybir.dt.bfloat16  # dtype of streamed x, weights, q/k, P, attnT, out
EXP = mybir.ActivationFunctionType.Exp


def _emit(nc, aps):
    xq, xk, xv = aps["xqT"], aps["xkT"], aps["xvT"]
    out_ap = aps["out"]

    with tile.TileContext(nc) as tc, \
         nc.allow_low_precision(reason="bf16 matmul input pipeline"):
        with tc.tile_pool(name="persist", bufs=1, space="SBUF") as sb, \
             tc.tile_pool(name="xres", bufs=4, space="SBUF") as xvp, \
             tc.tile_pool(name="xstream", bufs=3, space="SBUF") as xp, \
             tc.tile_pool(name="xstreamk", bufs=3, space="SBUF") as xkp, \
             tc.tile_pool(name="pexp", bufs=28, space="SBUF") as pa_pool, \
             tc.tile_pool(name="accsb", bufs=2, space="SBUF") as acs_pool, \
             tc.tile_pool(name="zrec", bufs=2, space="SBUF") as z_pool, \
             tc.tile_pool(name="normsb", bufs=8, space="SBUF") as nrm_pool, \
             tc.tile_pool(name="attnT", bufs=4, space="SBUF") as at_pool, \
             tc.tile_pool(name="obuf", bufs=3, space="SBUF") as ob_pool, \
             tc.tile_pool(name="sp", bufs=2, space="PSUM") as sp, \
             tc.tile_pool(name="accp", bufs=1, space="PSUM") as accp, \
             tc.tile_pool(name="ring", bufs=2, space="PSUM") as ring:

            wq_sb = sb.tile([128, 2048], XDT)
            wk_sb = sb.tile([128, 2048], XDT)
            wv_sb = sb.tile([128, 2048], XDT)
            wo_sb = sb.tile([128, 2048], XDT)
            bqT_sb = sb.tile([128, 2], F32)
            bkT_sb = sb.tile([128, 2], F32)
            qT_sb = sb.tile([128, 4096], XDT)
            kT_sb = sb.tile([128, 4096], XDT)
            vaug_sb = sb.tile([128, 16 * 260], XDT)
            ident = sb.tile([128, 128], XDT)

            masks.make_identity(nc, ident[:])
            # v_aug row layout per key-chunk j (260 cols): 4 x [v(64) 1];
            # the ones column accumulates Z alongside attn.V.
            onesF = sb.tile([128, 64], F32)
            nc.vector.memset(onesF[:], 1.0)
            vj = vaug_sb[:].rearrange("p (j r) -> p j r", r=260)
            for c in (64, 129, 194, 259):
                nc.vector.tensor_copy(vj[:, :, c:c + 1],
                                      onesF[:, 0:16].unsqueeze(2))
            # single-descriptor-batch weight loads: one dma_start each (the
            # descriptor-generation front end costs ~625ns per dma_start)
            nc.sync.dma_start(
                wq_sb[:].rearrange("p (d c) -> p d c", c=256),
                aps["wq"][:].rearrange("(d p) c -> p d c", p=128))
            nc.sync.dma_start(bqT_sb[:], aps["bqT"][:])

            # one [128,1024] slot = cc0|cc1 accumulators of one seq-quarter
            # sc; sp hosts sc0/sc1, accp sc2, ring sc3 (two 1-bank halves).
            def proj_slots(nm):
                s0 = sp.tile([128, 1024], F32, tag="s", name=f"{nm}s0")
                s1 = sp.tile([128, 1024], F32, tag="s", name=f"{nm}s1")
                s2 = accp.tile([128, 1024], F32, tag="acc", name=f"{nm}s2")
                r0 = ring.tile([128, 512], F32, tag="r", name=f"{nm}s3a")
                r1 = ring.tile([128, 512], F32, tag="r", name=f"{nm}s3b")
                # [cc][sc] -> accumulator AP
                return [[s0[:, 0:512], s1[:, 0:512], s2[:, 0:512], r0[:]],
                        [s0[:, 512:1024], s1[:, 512:1024], s2[:, 512:1024],
                         r1[:]]]

            def proj_stream(nm, x_ap, w_sb, ps):
                # 1MB chunks (two 128-row d-slices each): few DMA issues,
                # so the sync queue's ring credits never stall the stream.
                # q's first two chunks are half-size so its first matmuls
                # start ~1.5us sooner (the whole head chain shifts left).
                pool = xp if nm == "q" else xkp
                chunks = ([(0, 128), (128, 128), (256, 256), (512, 256),
                           (768, 256)] if nm == "q"
                          else [(0, 256), (256, 256), (512, 256),
                                (768, 256)])
                for ci, (r0, nr) in enumerate(chunks):
                    xt = pool.tile([128, nr // 128, 2048], XDT, tag="xs",
                                   name=f"x{nm}{ci}")
                    nc.sync.dma_start(
                        xt[:],
                        x_ap[r0:r0 + nr, :]
                        .rearrange("(c p) s -> p c s", p=128))
                    if ci == len(chunks) - 1:
                        # last chunk sc-major: sc0's accumulators stop
                        # first so their evac (which gates the next
                        # projection / the first scores) fires early
                        order = [(h, cc, sc) for sc in range(4)
                                 for cc in range(2)
                                 for h in range(nr // 128)]
                    else:
                        order = [(h, cc, sc) for h in range(nr // 128)
                                 for cc in range(2) for sc in range(4)]
                    for h, cc, sc in order:
                        d = r0 // 128 + h
                        nc.tensor.matmul(
                            ps[cc][sc],
                            w_sb[:, d * 256 + cc * 128:d * 256 + cc * 128 + 128],
                            xt[:, h, sc * 512:(sc + 1) * 512],
                            start=(d == 0), stop=(d == 7))

            def proj_evac(tT_sb, b_sb, ps, sc, cc0_dve=False):
                # cc0 on ACT and cc1 on DVE in parallel per seq-quarter;
                # cc0_dve routes cc0 to DVE too (GPSIMD can't read PSUM,
                # and ACT must stay clear for the exp stream)
                if cc0_dve:
                    nc.vector.tensor_scalar_add(
                        tT_sb[:, sc * 512:sc * 512 + 512],
                        ps[0][sc], b_sb[:, 0:1])
                else:
                    nc.scalar.add(tT_sb[:, sc * 512:sc * 512 + 512],
                                  ps[0][sc], b_sb[:, 0:1])
                nc.vector.tensor_scalar_add(
                    tT_sb[:, 2048 + sc * 512:2048 + sc * 512 + 512],
                    ps[1][sc], b_sb[:, 1:2])

            # ---- q then k projections (x streamed, DMA-paced) ----
            qps = proj_slots("qp")
            # dummy fp32 matmuls (~213ns each) ramp the PE clock
            # (0.65->2.4GHz after ~3us of continuous execution) and keep PE
            # busy until wq + the first x chunk land (~5.8us), so the real
            # projections start at full clock
            for i in range(38):
                nc.tensor.matmul(qps[i % 2][(i // 2) % 4][0:64, 0:64],
                                 onesF[:, 0:64],
                                 onesF[:, 0:64], start=True, stop=True)
            proj_stream("q", xq, wq_sb, qps)
            nc.sync.dma_start(
                wk_sb[:].rearrange("p (d c) -> p d c", c=256),
                aps["wk"][:].rearrange("(d p) c -> p d c", p=128))
            nc.sync.dma_start(bkT_sb[:], aps["bkT"][:])
            # q evacs BEFORE the k slots claim the same PSUM (slot WAR)
            for sc in range(4):
                proj_evac(qT_sb, bqT_sb, qps, sc)
            kps = proj_slots("kp")
            proj_stream("k", xk, wk_sb, kps)
            nc.sync.dma_start(
                wv_sb[:].rearrange("p (d c) -> p d c", c=256),
                aps["wv"][:].rearrange("(d p) c -> p d c", p=128))

            # ---- attention (exp-paced) with v projection woven in ----
            blocks = [(p, Q) for Q in range(4) for p in range(2)]

            # deferred PE-side work, drained <=2 PE-units per chunk so
            # scores (and ACT) never stall behind catch-up work
            chunk_no = [0]
            queue = []  # (push_chunk, pe_cost, closure)
            tail_mode = [False]

            def push(cost, fn, age=1):
                queue.append((chunk_no[0], cost, fn, age))

            def drain(budget, flush=False):
                while queue:
                    pc, cost, fn, age = queue[0]
                    if not flush and (pc > chunk_no[0] - age
                                      or cost > budget):
                        break
                    queue.pop(0)
                    fn()
                    budget -= cost

            # attnV accum subtile column base inside the padded 2-bank
            # tile (no slice may cross the 512-f32 bank boundary)
            def aoff(qs, h):
                return (qs % 2) * 130 + h * 65 + (qs // 2) * 512

            state = {}

            def attnv(bi, p, j):
                st = state[bi]
                if st[0] is None:
                    st[0] = accp.tile([128, 1024], F32, tag="acc",
                                      name=f"acc{bi}")
                acc, pBigs = st
                for qs in range(4):
                    for h in range(2):
                        va = j * 260 + (p * 2 + h) * 65
                        o = aoff(qs, h)
                        # start lazily zeroes the WHOLE 2KB psum bank, so
                        # only the first subtile per bank starts the group
                        # and only the last one closes it; the others land
                        # on pending-zero bytes (read as 0) at j==0
                        first = (qs, h) in ((0, 0), (2, 0))
                        last = (qs, h) in ((1, 1), (3, 1))
                        nc.tensor.matmul(
                            acc[:, o:o + 65],
                            pBigs[j][:, h * 512 + qs * 128:h * 512 + qs * 128 + 128],
                            vaug_sb[:, va:va + 65],
                            start=(j == 0 and first), stop=(j == 15 and last))

            def evac(bi):
                # free the accum slot fast: DVE and Pool copy one bank each
                acc = state[bi][0]
                acs = acs_pool.tile([128, 520], F32, tag="acs",
                                    name=f"acs{bi}")
                nc.vector.tensor_copy(acs[:, 0:260], acc[:, 0:260])
                nc.vector.tensor_copy(acs[:, 260:520], acc[:, 512:772])
                zr = z_pool.tile([128, 8], F32, tag="zr", name=f"zr{bi}")
                gv = acs[:].rearrange("p (g c) -> p g c", c=65)
                nc.vector.reciprocal(zr[:].unsqueeze(2), gv[:, :, 64:65])
                state[bi] = (acs, zr)

            def norm(bi):
                acs, zr = state[bi]
                nrms = []
                for qs in range(4):
                    nrm = nrm_pool.tile([128, 128], XDT, tag="nrm",
                                        name=f"nrm{bi}_{qs}")
                    for h in range(2):
                        g = qs * 2 + h
                        nc.vector.tensor_scalar_mul(
                            nrm[:, h * 64:(h + 1) * 64],
                            acs[:, g * 65:g * 65 + 64], zr[:, g:g + 1])
                    nrms.append(nrm)
                state[bi] = nrms

            def tpose(bi, p, Q):
                nrms = state.pop(bi)
                atT = at_pool.tile([128, 512], XDT, tag="at", name=f"at{bi}")
                for qs in range(4):
                    tp = ring.tile([128, 128], XDT, tag="r",
                                   name=f"tp{bi}_{qs}")
                    nc.tensor.transpose(tp[:], nrms[qs][:], ident[:])
                    nc.vector.tensor_copy(atT[:, qs * 128:(qs + 1) * 128],
                                          tp[:])
                state[("at", p, Q)] = atT

            def oproj(Q, t):
                atT0 = state[("at", 0, Q)]
                atT1 = state[("at", 1, Q)]
                if True:
                    ob = ob_pool.tile([128, 1024], XDT, tag="ob",
                                      name=f"ob{Q}_{t}")
                    for n in range(2):
                        op = ring.tile([128, 512], F32, tag="r",
                                       name=f"op{Q}_{t}_{n}")
                        nc.tensor.matmul(
                            op[:], atT0[:, t * 128:(t + 1) * 128],
                            wo_sb[:, n * 512:n * 512 + 512],
                            start=True, stop=False)
                        nc.tensor.matmul(
                            op[:], atT1[:, t * 128:(t + 1) * 128],
                            wo_sb[:, 1024 + n * 512:1024 + n * 512 + 512],
                            start=False, stop=True)
                        # the tail's n0 evacs go to ACT (idle there)
                        if n == 0 and tail_mode[0]:
                            nc.scalar.copy(ob[:, 0:512], op[:])
                        else:
                            nc.vector.tensor_copy(
                                ob[:, n * 512:(n + 1) * 512], op[:])
                        nc.sync.dma_start(
                            out_ap[Q * 512 + t * 128:Q * 512 + (t + 1) * 128,
                                   n * 512:(n + 1) * 512],
                            ob[:, n * 512:(n + 1) * 512])

            all_chunks = [(bi, p, Q, j)
                          for bi, (p, Q) in enumerate(blocks)
                          for j in range(16)]
            chunk_iter = iter(all_chunks)

            def emit_next():
                bi, p, Q, j = next(chunk_iter)
                if j == 0:
                    state[bi] = [None, [None] * 16]
                # no drains through blocks 0-1: the first attnV drains
                # would stall PE on the not-yet-finished v evacs
                emit_chunk(bi, p, Q, j,
                           budget=(0 if chunk_no[0] < 20 else
                                   2 if chunk_no[0] < 28 else 4))

            def emit_chunk(bi, p, Q, j, budget=2):
                qb = p * 2048 + Q * 512
                kb = p * 2048 + j * 128
                sBig = sp.tile([128, 1024], F32, tag="s", name=f"s{bi}_{j}")
                nc.tensor.matmul(sBig[:, 0:512],
                                 kT_sb[0:64, kb:kb + 128],
                                 qT_sb[0:64, qb:qb + 512],
                                 start=True, stop=True)
                nc.tensor.matmul(sBig[:, 512:1024],
                                 kT_sb[64:128, kb:kb + 128],
                                 qT_sb[64:128, qb:qb + 512],
                                 start=True, stop=True)
                pBig = pa_pool.tile([128, 1024], XDT, tag="pa",
                                    name=f"pb{bi}_{j}")
                nc.scalar.activation(pBig[:], sBig[:], EXP, scale=0.125)
                state[bi][1][j] = pBig
                push(2, lambda bi=bi, p=p, j=j: attnv(bi, p, j),
                 age=2 if j == 0 else 1)
                if j == 15 and bi != 7:  # bi 7 gets the fine-grained tail
                    push(0, lambda bi=bi: evac(bi))
                    push(0, lambda bi=bi: norm(bi))
                    push(3, lambda bi=bi, p=p, Q=Q: tpose(bi, p, Q))
                    if p == 1:
                        for t in range(4):
                            push(4, lambda Q=Q, t=t: oproj(Q, t))
                drain(budget)
                chunk_no[0] += 1

            # k evacs interleaved with the first chunks so ACT reaches
            # exp(0,0) right after the sc0 evac
            # sc0 on ACT so exp(0,0) follows immediately; the rest of cc0
            # on DVE so they don't queue ahead of exps on ACT
            proj_evac(kT_sb, bkT_sb, kps, 0)
            emit_next()
            proj_evac(kT_sb, bkT_sb, kps, 1, cc0_dve=True)
            emit_next()
            proj_evac(kT_sb, bkT_sb, kps, 2, cc0_dve=True)
            proj_evac(kT_sb, bkT_sb, kps, 3, cc0_dve=True)

            # v projection: 8 one-bank-half accum targets per half-pass,
            # borrowed from accp (4) + ring (2+2); chunks woven in
            xvt = []
            for half in range(2):
                vs2 = accp.tile([128, 1024], F32, tag="acc",
                                name=f"vs{half}a")
                vr0 = ring.tile([128, 512], F32, tag="r", name=f"vs{half}b")
                vr1 = ring.tile([128, 512], F32, tag="r", name=f"vs{half}c")
                vps = [vs2[:, 0:256], vs2[:, 256:512],
                       vs2[:, 512:768], vs2[:, 768:1024],
                       vr0[:, 0:256], vr0[:, 256:512],
                       vr1[:, 0:256], vr1[:, 256:512]]
                for dc in range(4):
                    emit_next()
                    if half == 0:
                        xt = xvp.tile([128, 2, 2048], XDT, tag="xv",
                                      name=f"xv{dc}")
                        nc.sync.dma_start(
                            xt[:],
                            xv[dc * 256:(dc + 1) * 256, :]
                            .rearrange("(c p) s -> p c s", p=128))
                        xvt.append(xt)
                    for h in range(2):
                        if half == 1:
                            emit_next()
                        d = dc * 2 + h
                        for i in range(8):
                            jj = half * 8 + i
                            # two v targets share each bank: only i-even
                            # starts the group, only i-odd closes it
                            nc.tensor.matmul(
                                vps[i],
                                xvt[dc][:, h, jj * 128:(jj + 1) * 128],
                                wv_sb[:, d * 256:(d + 1) * 256],
                                start=(d == 0 and i % 2 == 0),
                                stop=(d == 7 and i % 2 == 1))
                    if half == 0:
                        emit_next()
                for i in range(8):
                    jj = half * 8 + i
                    base = jj * 260
                    dst = vaug_sb[:, base:base + 260] \
                        .rearrange("p (g c) -> p g c", c=65)[:, :, 0:64]
                    nc.vector.tensor_copy(
                        dst, vps[i].rearrange("p (g c) -> p g c", c=64))
            nc.sync.dma_start(
                wo_sb[:].rearrange("p (d c) -> p d c", c=2048),
                aps["wo"][:].rearrange("p (d c) -> p d c", c=2048))

            # remaining chunks: the queue drains the deferred work
            for _ in range(128 - 22):
                emit_next()
            # fine-grained per-qsub tail for the last block: each qsub is
            # normalized, transposed, projected and stored independently so
            # the engines pipeline instead of running stage-by-stage
            tail_mode[0] = True
            drain(0, flush=True)
            bi, (p, Q) = 7, blocks[7]
            acc = state[bi][0]
            zr = z_pool.tile([128, 8], F32, tag="zr", name=f"zr{bi}")
            atT0 = state[("at", 0, Q)]
            # all 1/Z up front (straight from accum PSUM), then a per-qsub
            # mul -> transpose -> outproj -> store pipeline across engines
            for bank, base in ((0, 0), (1, 512)):
                gvv = acc[:, base:base + 260] \
                    .rearrange("p (g c) -> p g c", c=65)
                nc.vector.reciprocal(
                    zr[:, bank * 4:bank * 4 + 4].unsqueeze(2),
                    gvv[:, :, 64:65])
            # stage-major: all norm->transpose->atT pipelines first (DVE ->
            # PE -> ACT back-to-back), then all outproj matmuls + stores, so
            # no engine round-robins between stages with semaphore hops
            # one strided broadcast-multiply per bank normalizes 4 groups
            # at once (in1 rides a zero-stride view of 1/Z)
            nrmall = nrm_pool.tile([128, 512], XDT, tag="nrmall",
                                   name=f"nrmall{bi}")
            for bank, base in ((0, 0), (1, 512)):
                gin = acc[:, base:base + 260]                     .rearrange("p (g c) -> p g c", c=65)[:, :, 0:64]
                gz = zr[:, bank * 4:bank * 4 + 4].unsqueeze(2)                     .to_broadcast([128, 4, 64])
                gout = nrmall[:, bank * 256:bank * 256 + 256]                     .rearrange("p (g c) -> p g c", c=64)
                nc.vector.tensor_mul(gout, gin, gz)
            atTs = []
            for qs in range(4):
                nrm = nrmall[:, qs * 128:(qs + 1) * 128]
                tp = ring.tile([128, 128], XDT, tag="r", name=f"tp{bi}_{qs}")
                nc.tensor.transpose(tp[:], nrm, ident[:])
                atT = at_pool.tile([128, 128], XDT, tag="att",
                                   name=f"att{bi}_{qs}")
                nc.scalar.copy(atT[:], tp[:])
                atTs.append(atT)
            for t in range(4):
                ob = ob_pool.tile([128, 1024], XDT, tag="ob",
                                  name=f"ob{Q}_{t}")
                for n in range(2):
                    op = ring.tile([128, 512], F32, tag="r",
                                   name=f"op{Q}_{t}_{n}")
                    nc.tensor.matmul(
                        op[:], atT0[:, t * 128:(t + 1) * 128],
                        wo_sb[:, n * 512:n * 512 + 512],
                        start=True, stop=False)
                    nc.tensor.matmul(
                        op[:], atTs[t][:],
                        wo_sb[:, 1024 + n * 512:1024 + n * 512 + 512],
                        start=False, stop=True)
                    if n == 0:
                        nc.scalar.copy(ob[:, 0:512], op[:])
                    else:
                        nc.vector.tensor_copy(ob[:, 512:1024], op[:])
                    nc.sync.dma_start(
                        out_ap[Q * 512 + t * 128:Q * 512 + (t + 1) * 128,
                               n * 512:(n + 1) * 512],
                        ob[:, n * 512:(n + 1) * 512])


_NC = None


def _get_nc():
    global _NC
    if _NC is None:
        nc = bacc.Bacc("TRN2", target_bir_lowering=False, debug=False,
                       enable_asserts=False, num_devices=8)
        aps = {}
        for nm, shp in [("xqT", (D, S)), ("xkT", (D, S)), ("xvT", (D, S)),
                        ("wq", (D, 256)), ("wk", (D, 256)), ("wv", (D, 256)),
                        ("wo", (128, 2048)), ("out", (S, D))]:
            kind = "ExternalOutput" if nm == "out" else "ExternalInput"
            aps[nm] = nc.dram_tensor(nm, shp, XDT, kind=kind).ap()
        for nm, shp in [("bqT", (128, 2)), ("bkT", (128, 2))]:
            aps[nm] = nc.dram_tensor(nm, shp, F32, kind="ExternalInput").ap()
        _emit(nc, aps)
        nc.compile()
        _NC = nc
    return _NC


def _run(inputs, trace=False):
    nc = _get_nc()
    f = np.float32
    bf = ml_dtypes.bfloat16
    q = np.asarray(inputs["query"], dtype=f)
    k = np.asarray(inputs["key"], dtype=f)
    v = np.asarray(inputs["value"], dtype=f)
    Wq = np.asarray(inputs["Wq"], dtype=f)
    Wk = np.asarray(inputs["Wk"], dtype=f)
    Wv = np.asarray(inputs["Wv"], dtype=f)
    Wo = np.asarray(inputs["Wo"], dtype=f)
    bq = np.asarray(inputs["bq"], dtype=f)
    bk = np.asarray(inputs["bk"], dtype=f)
    bv = np.asarray(inputs["bv"], dtype=f)
    bo = np.asarray(inputs["bo"], dtype=f)

    xT = {b: (np.ascontiguousarray(q[b].T).astype(bf),
              np.ascontiguousarray(k[b].T).astype(bf),
              np.ascontiguousarray(v[b].T).astype(bf)) for b in range(B)}
    in_maps = []
    for i in range(8):
        b, hg = divmod(i, 4)
        c0 = hg * 256
        woc = Wo[c0:c0 + 256, :]
        in_maps.append({
            "xqT": xT[b][0], "xkT": xT[b][1], "xvT": xT[b][2],
            "wq": np.ascontiguousarray(Wq[:, c0:c0 + 256]).astype(bf),
            "wk": np.ascontiguousarray(Wk[:, c0:c0 + 256]).astype(bf),
            "wv": np.ascontiguousarray(Wv[:, c0:c0 + 256]).astype(bf),
            "bqT": np.ascontiguousarray(bq[c0:c0 + 256].reshape(2, 128).T),
            "bkT": np.ascontiguousarray(bk[c0:c0 + 256].reshape(2, 128).T),
            "wo": np.ascontiguousarray(
                np.concatenate([woc[0:128, :], woc[128:256, :]],
                               axis=1)).astype(bf),
        })

    res = bass_utils.run_bass_kernel_spmd(nc, in_maps, core_ids=list(range(8)),
                                          trace=trace)
    out = np.zeros((B, S, D), dtype=f)
    for i in range(8):
        out[i // 4] += np.asarray(res.results[i]["out"], dtype=f)
    out += (bv @ Wo + bo)[None, None, :]
    return out, res


def kernel(**inputs):
    out, _ = _run(inputs, trace=False)
    return out
